# revision 5
# baseline (speedup 1.0000x reference)
"""Bahdanau additive-attention kernel for Trainium2, 8 NeuronCores.

Problem (B=32, S=2048, H=1024, E=2H):
    hid_proj = hidden @ w_h.T + attn_b                  # (B, H)
    enc_proj[b,s,h] = sum_e enc[b,s,e] * w_e[h,e]       # (B, S, H)   <- dominant
    energy = tanh(hid_proj[:,None,:] + enc_proj)
    scores[b,s] = sum_h energy[b,s,h] * v_w[h]
    attw = softmax(scores, axis=1)
    context[b,e] = sum_s attw[b,s] * enc[b,s,e]

Sharding: data-parallel over batch, 4 batches per core.

The dominant enc_proj GEMM runs in fp8e4 (TRN e4m3, max 240) with
MatmulPerfMode.DoubleRow: K=256 contracted per instruction at 0.5 PE
cycles per output row -> 2x bf16 throughput (109us/core vs 437us).

fp8 quantization of enc and w_e alone costs ~1.9e-2 max-rel error on the
attention weights (threshold 2e-2).  To recover accuracy at almost no PE
cost, the score error is corrected to FIRST ORDER: with pre = hid + enc@w,
  dS[b,s] ~= sum_h v_h tanh'(pre_bhs) * (elo.w_h + enc8.dw_h)
Approximating tanh'(pre_bhs) by its per-(b,h) expectation c_bh (computable
on the host by a Gauss-Hermite integral, since pre ~ N(hid_proj_bh,
||w_h||^2) for unit-variance enc), both terms collapse into per-batch
rank-1 GEMVs:
  dS ~= elo . u_b + enc8 . r_b,   u_b = sum_h v_h c_bh w_h,
                                  r_b = sum_h v_h c_bh dw_h
which the kernel accumulates straight into the score PSUM as cheap fp8
DoubleRow matmuls against the already-streaming enc8 tile and a second
fp8 stream of the quantization residual elo = fp8(16*(enc - enc8)).
Measured (numpy sim): attw err 1.9e-2 -> 6.7e-3, ctx 2.6e-3.

Scales (all folded into ACT activation scales; stored operand = natural
value x scale): w8 = 256 x w_e (w values ~1e-2 would be fp8-denormal),
elo = 16 x residual, v = 1024 x v_w (bf16), r = 1024 x r_nat,
u = 64 x u_nat.  Score PSUM is 1024 x natural -> exp(scale=1/1024);
main-GEMM PSUM is 256 x natural -> tanh(scale=1/256, bias=hid_proj).

The v-dot (scores) runs in bf16 (fp8 energy costs ~1e-2 attw error) and
phase 2 (context) runs in bf16 (fp8 enc there costs ~2.5e-2 ctx error:
quantization noise does not average down against a weighted mean).

All DRAM layouts are pre-transposed on the host so every DMA is a plain
contiguous per-partition copy (no xbar transpose, 8-16KB per partition
per descriptor).

Per-core PE roofline: 109 (main fp8 DR) + 27 (corrections) + 27 (v-dot
bf16) + 55 (context bf16) + 7 (PE transposes) ~= 225us.  DMA: 16.8MB
enc8T + 16.8MB eloT + 33.5MB enc bf16 natural = 67MB ~= 187us @ 358GB/s,
overlapped under PE.
"""

import numpy as np
import ml_dtypes

import concourse.bass as bass
import concourse.tile as tile
import concourse.mybir as mybir
from concourse import bacc
from concourse.bass_utils import run_bass_kernel_spmd

B, S, H = 32, 2048, 1024
E = 2 * H
NCORES = 8
BL = B // NCORES          # batches per core
P = 128                   # partitions
KC = E // P               # 16 contraction chunks
KP = KC // 2              # 8 DoubleRow k-pairs
MC = H // P               # 8 h chunks
NT = 512                  # moving free-dim per matmul (1 PSUM bank of fp32)
ST = S // NT              # 4 s-tiles per batch
SJ = S // P               # 16 s-128 blocks per batch
QT = NT // P              # 4 s-128 blocks per s-tile

F32 = mybir.dt.float32
BF16 = mybir.dt.bfloat16
FP8 = mybir.dt.float8e4
BF16_NP = ml_dtypes.bfloat16
FP8_NP = ml_dtypes.float8_e4m3
DR = mybir.MatmulPerfMode.DoubleRow

SC_W = 256.0              # w8 stored scale
SC_EL = 16.0              # elo stored scale
SC_S = 1024.0             # score-PSUM scale (v, r stored x1024; u x64)


def build_nc(reps=1):
    nc = bacc.Bacc("TRN2", target_bir_lowering=False, debug=False)

    t8_in = nc.dram_tensor("t8", [BL * ST, P, KC, NT], FP8, kind="ExternalInput")
    elo_in = nc.dram_tensor("elo", [BL * ST, P, KC, NT], FP8, kind="ExternalInput")
    nat_in = nc.dram_tensor("nat", [BL * ST, P, QT, E], BF16, kind="ExternalInput")
    w_in = nc.dram_tensor("w8", [P, KC, H], FP8, kind="ExternalInput")
    v_in = nc.dram_tensor("v1k", [P, MC], BF16, kind="ExternalInput")
    hid_in = nc.dram_tensor("hidc", [P, BL, MC], F32, kind="ExternalInput")
    r_in = nc.dram_tensor("rcol", [P, KC, BL, 16], FP8, kind="ExternalInput")
    u_in = nc.dram_tensor("ucol", [P, KC, BL, 16], FP8, kind="ExternalInput")
    ctx_out = nc.dram_tensor("ctx", [BL, E], F32, kind="ExternalOutput")
    attw_out = nc.dram_tensor("attw", [BL, S], F32, kind="ExternalOutput")

    with tile.TileContext(nc) as tc:
        with (
            tc.tile_pool(name="singles", bufs=1) as singles,
            tc.tile_pool(name="pT", bufs=2) as pT,
            tc.tile_pool(name="pL", bufs=2) as pL,
            tc.tile_pool(name="pen", bufs=2) as pen,
            tc.tile_pool(name="pnt", bufs=2) as pnt,
            tc.tile_pool(name="prow", bufs=1) as prow,
            tc.tile_pool(name="pwcol", bufs=2) as pwcol,
            tc.tile_pool(name="pmm", bufs=2, space="PSUM") as pmm,
            tc.tile_pool(name="psc", bufs=1, space="PSUM") as psc,
            tc.tile_pool(name="pwc", bufs=1, space="PSUM") as pwc,
            tc.tile_pool(name="pctx", bufs=1, space="PSUM") as pctx,
        ):
            # resident small operands
            w_sb = singles.tile([P, KC, H], FP8)
            nc.sync.dma_start(out=w_sb, in_=w_in[:, :, :])
            v_sb = singles.tile([P, MC], BF16)
            nc.sync.dma_start(out=v_sb, in_=v_in[:, :])
            hid_sb = singles.tile([P, BL, MC], F32)
            nc.sync.dma_start(out=hid_sb, in_=hid_in[:, :, :])
            r_sb = singles.tile([P, KC, BL, 16], FP8)
            nc.sync.dma_start(out=r_sb, in_=r_in[:, :, :, :])
            u_sb = singles.tile([P, KC, BL, 16], FP8)
            nc.sync.dma_start(out=u_sb, in_=u_in[:, :, :, :])
            ident = singles.tile([1, 1], F32)
            nc.vector.memset(ident, 1.0)

            for rep_b in range(reps * BL):
                rep, b = divmod(rep_b, BL)
                ex_row = prow.tile([1, S], F32)
                zq = prow.tile([1, ST], F32)
                wcols = pwcol.tile([P, SJ], BF16)
                for st in range(ST):
                    T8 = pT.tile([P, KC, NT], FP8)
                    nc.sync.dma_start(out=T8, in_=t8_in[b * ST + st])
                    TL = pL.tile([P, KC, NT], FP8)
                    nc.sync.dma_start(out=TL, in_=elo_in[b * ST + st])

                    en = pen.tile([P, MC, NT], BF16)
                    sc = psc.tile([16, NT], F32)
                    # first-order score corrections: enc8.r_b + elo.u_b
                    for kp in range(KP):
                        nc.tensor.matmul(
                            sc,
                            lhsT=r_sb[:, 2 * kp : 2 * kp + 2, b, :],
                            rhs=T8[:, 2 * kp : 2 * kp + 2, :],
                            start=(kp == 0),
                            stop=False,
                            perf_mode=DR,
                        )
                    for kp in range(KP):
                        nc.tensor.matmul(
                            sc,
                            lhsT=u_sb[:, 2 * kp : 2 * kp + 2, b, :],
                            rhs=TL[:, 2 * kp : 2 * kp + 2, :],
                            start=False,
                            stop=False,
                            perf_mode=DR,
                        )
                    # main GEMM, fp8 DoubleRow, K=2048 PSUM-accumulated
                    for m in range(MC):
                        ps = pmm.tile([P, NT], F32)
                        for kp in range(KP):
                            nc.tensor.matmul(
                                ps,
                                lhsT=w_sb[:, 2 * kp : 2 * kp + 2, m * P : (m + 1) * P],
                                rhs=T8[:, 2 * kp : 2 * kp + 2, :],
                                start=(kp == 0),
                                stop=(kp == KP - 1),
                                perf_mode=DR,
                            )
                        nc.scalar.activation(
                            out=en[:, m, :],
                            in_=ps,
                            func=mybir.ActivationFunctionType.Tanh,
                            bias=hid_sb[:, b, m : m + 1],
                            scale=1.0 / SC_W,
                        )
                    # v-dot in bf16
                    for m in range(MC):
                        nc.tensor.matmul(
                            sc[0:1, :],
                            lhsT=v_sb[:, m : m + 1],
                            rhs=en[:, m, :],
                            start=False,
                            stop=(m == MC - 1),
                            skip_group_check=True,
                        )
                    # exp segment straight from PSUM + partial Z
                    nc.scalar.activation(
                        out=ex_row[:, st * NT : (st + 1) * NT],
                        in_=sc[0:1, :],
                        func=mybir.ActivationFunctionType.Exp,
                        scale=1.0 / SC_S,
                        accum_out=zq[:, st : st + 1],
                    )
                    # transpose the fresh exp segment into bf16 columns
                    for q in range(QT):
                        j = st * QT + q
                        pw = pwc.tile([P, 1], F32)
                        nc.tensor.transpose(pw, ex_row[:, j * P : (j + 1) * P], ident)
                        nc.vector.tensor_copy(out=wcols[:, j : j + 1], in_=pw)

                # Z = sum of the 4 segment accums; rz = 1/Z
                zt = prow.tile([1, 1], F32)
                zc = prow.tile([1, ST], F32)
                nc.scalar.activation(
                    out=zc,
                    in_=zq,
                    func=mybir.ActivationFunctionType.Identity,
                    accum_out=zt,
                )
                rz = prow.tile([1, 1], F32)
                nc.vector.reciprocal(out=rz, in_=zt)

                # attention-weight output row (off the context critical path)
                wrow = prow.tile([1, S], F32)
                nc.scalar.activation(
                    out=wrow,
                    in_=ex_row,
                    func=mybir.ActivationFunctionType.Identity,
                    scale=rz,
                )
                nc.sync.dma_start(out=attw_out[b], in_=wrow)

                # context: ctx[e] = (sum_s exp[s] * enc[s, e]) / Z, bf16
                cps = [
                    pctx.tile([1, NT], F32, name=f"cps{g}_{b}_{rep}", tag=f"cps{g}")
                    for g in range(4)
                ]
                for st in range(ST):
                    nt = pnt.tile([P, QT, E], BF16)
                    nc.sync.dma_start(out=nt, in_=nat_in[b * ST + st])
                    for q in range(QT):
                        j = st * QT + q
                        for g in range(4):
                            nc.tensor.matmul(
                                cps[g],
                                lhsT=wcols[:, j : j + 1],
                                rhs=nt[:, q, g * NT : (g + 1) * NT],
                                start=(j == 0),
                                stop=(j == SJ - 1),
                            )
                ctxrow = prow.tile([1, E], F32)
                for g in range(4):
                    nc.vector.tensor_scalar_mul(
                        ctxrow[:, g * NT : (g + 1) * NT], cps[g], rz
                    )
                nc.sync.dma_start(out=ctx_out[b], in_=ctxrow)

    nc.compile()
    return nc


_CACHE = {}


def _get_nc():
    if "nc" not in _CACHE:
        _CACHE["nc"] = build_nc()
    return _CACHE["nc"]


def prep_in_maps(hidden, encoder_outputs, attn_w, attn_b, v_w):
    hidden = np.asarray(hidden, dtype=np.float32)
    enc = np.asarray(encoder_outputs, dtype=np.float32)
    attn_w = np.asarray(attn_w, dtype=np.float32)
    attn_b = np.asarray(attn_b, dtype=np.float32)
    v_w = np.asarray(v_w, dtype=np.float32)

    w_h = attn_w[:, :H]
    w_e = attn_w[:, H:]
    hid_proj = (hidden @ w_h.T + attn_b).astype(np.float32)   # (B, H) exact

    w8 = (w_e * SC_W).astype(FP8_NP)                           # (H, E)
    dw = w_e - w8.astype(np.float32) / SC_W                    # exact residual

    # c[b,h] = E[tanh'(hid_proj_bh + sigma_h Z)] via Gauss-Hermite
    sigma = np.linalg.norm(w_e, axis=1)
    nodes, wts = np.polynomial.hermite.hermgauss(20)
    muz = (
        hid_proj[:, :, None]
        + np.sqrt(2.0) * sigma[None, :, None] * nodes[None, None, :]
    )
    c = (wts * (1.0 - np.tanh(muz) ** 2)).sum(-1) / np.sqrt(np.pi)  # (B, H)

    u_nat = np.einsum("h,bh,he->be", v_w, c, w_e, optimize=True)
    r_nat = np.einsum("h,bh,he->be", v_w, c, dw, optimize=True)
    u8 = (u_nat * (SC_S / SC_EL)).astype(FP8_NP)               # (B, E)
    r8 = (r_nat * SC_S).astype(FP8_NP)

    enc8 = enc.astype(FP8_NP)
    elo = ((enc - enc8.astype(np.float32)) * SC_EL).astype(FP8_NP)
    encb = enc.astype(BF16_NP)

    wT = np.ascontiguousarray(w8.T.reshape(KC, P, H).transpose(1, 0, 2))
    vcol = np.ascontiguousarray((v_w * SC_S).reshape(MC, P).T).astype(BF16_NP)

    in_maps = []
    for cix in range(NCORES):
        sl = slice(cix * BL, (cix + 1) * BL)
        hp = hid_proj[sl]
        hidc = np.ascontiguousarray(
            hp.reshape(BL, MC, P).transpose(2, 0, 1)
        ).astype(np.float32)
        # t8[b*ST+st, p, k, n] = enc8[b, st*NT+n, k*P+p]
        t8 = np.ascontiguousarray(
            enc8[sl].reshape(BL, ST, NT, KC, P).transpose(0, 1, 4, 3, 2)
        ).reshape(BL * ST, P, KC, NT)
        el = np.ascontiguousarray(
            elo[sl].reshape(BL, ST, NT, KC, P).transpose(0, 1, 4, 3, 2)
        ).reshape(BL * ST, P, KC, NT)
        # nat[b*ST+st, p, q, e] = encb[b, st*NT + q*P + p, e]
        nat = np.ascontiguousarray(
            encb[sl].reshape(BL, ST, QT, P, E).transpose(0, 1, 3, 2, 4)
        ).reshape(BL * ST, P, QT, E)
        rcol = np.zeros((P, KC, BL, 16), dtype=FP8_NP)
        rcol[:, :, :, 0] = r8[sl].reshape(BL, KC, P).transpose(2, 1, 0)
        ucol = np.zeros((P, KC, BL, 16), dtype=FP8_NP)
        ucol[:, :, :, 0] = u8[sl].reshape(BL, KC, P).transpose(2, 1, 0)
        in_maps.append(
            {
                "t8": t8,
                "elo": el,
                "nat": nat,
                "w8": wT,
                "v1k": vcol,
                "hidc": hidc,
                "rcol": rcol,
                "ucol": ucol,
            }
        )
    return in_maps


def kernel(hidden, encoder_outputs, attn_w, attn_b, v_w):
    in_maps = prep_in_maps(hidden, encoder_outputs, attn_w, attn_b, v_w)
    nc = _get_nc()
    res = run_bass_kernel_spmd(nc, in_maps, core_ids=list(range(NCORES)))
    ctx = np.concatenate([res.results[c]["ctx"] for c in range(NCORES)], axis=0)
    attw = np.concatenate([res.results[c]["attw"] for c in range(NCORES)], axis=0)
    return ctx.astype(np.float32), attw.astype(np.float32)


# revision 7
# speedup vs baseline: 1.0632x; 1.0632x over previous
"""Bahdanau additive-attention kernel for Trainium2, 8 NeuronCores.

Problem (B=32, S=2048, H=1024, E=2H):
    hid_proj = hidden @ w_h.T + attn_b                  # (B, H)
    enc_proj[b,s,h] = sum_e enc[b,s,e] * w_e[h,e]       # (B, S, H)   <- dominant
    energy = tanh(hid_proj[:,None,:] + enc_proj)
    scores[b,s] = sum_h energy[b,s,h] * v_w[h]
    attw = softmax(scores, axis=1)
    context[b,e] = sum_s attw[b,s] * enc[b,s,e]

Sharding: data-parallel over batch, 4 batches per core.

All heavy matmuls run in fp8e4 (TRN e4m3, max 240) with
MatmulPerfMode.DoubleRow: K=256 contracted per instruction at 0.5 PE
cycles per output row -> 2x bf16 throughput.

fp8 quantization of enc and w_e alone costs ~1.9e-2 max-rel error on the
attention weights (threshold 2e-2).  To recover accuracy at almost no PE
cost, the score error is corrected to FIRST ORDER: with pre = hid + enc@w,
  dS[b,s] ~= sum_h v_h tanh'(pre_bhs) * (elo.w_h + enc8.dw_h)
Approximating tanh'(pre_bhs) by its per-(b,h) expectation c_bh (computable
on the host by a Gauss-Hermite integral, since pre ~ N(hid_proj_bh,
||w_h||^2) for unit-variance enc), both terms collapse into per-batch
rank-1 GEMVs:
  dS ~= elo . u_b + enc8 . r_b,   u_b = sum_h v_h c_bh w_h,
                                  r_b = sum_h v_h c_bh dw_h
which the kernel accumulates straight into the score PSUM as cheap fp8
DoubleRow matmuls against the already-streaming enc8 tile and a second
fp8 stream of the quantization residual elo = fp8(enc - enc8).
Measured (numpy sim): attw err 1.9e-2 -> 6.7e-3, ctx 2.1e-3.

The context matmul runs as a 3-term fp8 hi/lo split (exact to 2nd order
because context is linear in enc):
  ctx = (w8c + wlo).enc8 + w8c.elo,  w8c = fp8(exp(s)), wlo = fp8(exp-w8c)
giving ~0.1% effective operand precision (better than bf16) at 3/4 the
bf16 PE cost.  The v-dot stays bf16 (fp8 energy alone costs ~1e-2 attw).

Scores are drained to fp16 rows (10-bit mantissa; bf16 would add 4e-3
error), PE-transposed as fp16 (1 cycle/row vs 2 for fp32), and exp runs
per-column on ACT.  Softmax Z comes from the row-exp's free-dim accum.

Scales (stored operand = natural value x scale): w8 = 256 x w_e (w values
~1e-2 would be fp8-denormal), v = 1024 x v_w (bf16), r = 1024 x r_nat,
u = 1024 x u_nat; elo/enc8/exp tensors at natural scale.  Main-GEMM PSUM
is 256x natural -> tanh(scale=1/256, bias=hid_proj); score PSUM is 1024x
-> fp16 drain with scale=1/1024; context PSUM natural -> drain x (1/Z).

All DRAM layouts are pre-transposed on the host so every DMA is a plain
contiguous per-partition copy (no xbar transpose, 8KB per partition per
descriptor).

Per-core PE roofline: 109.2 (main fp8 DR) + 27.3 (corrections) + 27.3
(v-dot bf16) + 41 (context 3-term fp8 DR) + 3.4 (fp16 transposes)
~= 208us.  DMA: 4 fp8 streams (enc8T, eloT, enc8 natural, elo natural)
= 67MB ~= 187us @ 358GB/s, overlapped under PE.
"""

import numpy as np
import ml_dtypes

import concourse.bass as bass
import concourse.tile as tile
import concourse.mybir as mybir
from concourse import bacc
from concourse.bass_utils import run_bass_kernel_spmd

B, S, H = 32, 2048, 1024
E = 2 * H
NCORES = 8
BL = B // NCORES          # batches per core
P = 128                   # partitions
KC = E // P               # 16 contraction chunks
KP = KC // 2              # 8 DoubleRow k-pairs
MC = H // P               # 8 h chunks
NT = 512                  # moving free-dim per matmul (1 PSUM bank of fp32)
ST = S // NT              # 4 s-tiles per batch
SJ = S // P               # 16 s-128 blocks per batch
JP = SJ // 2              # 8 DoubleRow s-block pairs
QT = NT // P              # 4 s-128 blocks per s-tile

F32 = mybir.dt.float32
F16 = mybir.dt.float16
BF16 = mybir.dt.bfloat16
FP8 = mybir.dt.float8e4
BF16_NP = ml_dtypes.bfloat16
FP8_NP = ml_dtypes.float8_e4m3
DR = mybir.MatmulPerfMode.DoubleRow

SC_W = 256.0              # w8 stored scale
SC_S = 1024.0             # score-PSUM scale (v, r, u stored x1024)


def build_nc(reps=1):
    nc = bacc.Bacc("TRN2", target_bir_lowering=False, debug=False)

    t8_in = nc.dram_tensor("t8", [BL * ST, P, KC, NT], FP8, kind="ExternalInput")
    tl_in = nc.dram_tensor("tl", [BL * ST, P, KC, NT], FP8, kind="ExternalInput")
    n8_in = nc.dram_tensor("n8", [BL * ST, P, QT, E], FP8, kind="ExternalInput")
    nl_in = nc.dram_tensor("nl", [BL * ST, P, QT, E], FP8, kind="ExternalInput")
    w_in = nc.dram_tensor("w8", [P, KC, H], FP8, kind="ExternalInput")
    v_in = nc.dram_tensor("v1k", [P, MC], BF16, kind="ExternalInput")
    hid_in = nc.dram_tensor("hidc", [P, BL, MC], F32, kind="ExternalInput")
    r_in = nc.dram_tensor("rcol", [P, KC, BL, 16], FP8, kind="ExternalInput")
    u_in = nc.dram_tensor("ucol", [P, KC, BL, 16], FP8, kind="ExternalInput")
    ctx_out = nc.dram_tensor("ctx", [BL, E], F32, kind="ExternalOutput")
    attw_out = nc.dram_tensor("attw", [BL, S], F32, kind="ExternalOutput")

    with tile.TileContext(nc) as tc:
        with (
            tc.tile_pool(name="singles", bufs=1) as singles,
            tc.tile_pool(name="pT", bufs=2) as pT,
            tc.tile_pool(name="pL", bufs=2) as pL,
            tc.tile_pool(name="pen", bufs=2) as pen,
            tc.tile_pool(name="pn8", bufs=2) as pn8,
            tc.tile_pool(name="pnl", bufs=2) as pnl,
            tc.tile_pool(name="prow", bufs=1) as prow,
            tc.tile_pool(name="pwcol", bufs=2) as pwcol,
            tc.tile_pool(name="pmm", bufs=2, space="PSUM") as pmm,
            tc.tile_pool(name="psc", bufs=1, space="PSUM") as psc,
            tc.tile_pool(name="pwc", bufs=1, space="PSUM") as pwc,
            tc.tile_pool(name="pctx", bufs=1, space="PSUM") as pctx,
        ):
            # resident small operands
            w_sb = singles.tile([P, KC, H], FP8)
            nc.sync.dma_start(out=w_sb, in_=w_in[:, :, :])
            v_sb = singles.tile([P, MC], BF16)
            nc.sync.dma_start(out=v_sb, in_=v_in[:, :])
            hid_sb = singles.tile([P, BL, MC], F32)
            nc.sync.dma_start(out=hid_sb, in_=hid_in[:, :, :])
            r_sb = singles.tile([P, KC, BL, 16], FP8)
            nc.sync.dma_start(out=r_sb, in_=r_in[:, :, :, :])
            u_sb = singles.tile([P, KC, BL, 16], FP8)
            nc.sync.dma_start(out=u_sb, in_=u_in[:, :, :, :])
            ident16 = singles.tile([1, 1], F16)
            nc.vector.memset(ident16, 1.0)

            for rep_b in range(reps * BL):
                rep, b = divmod(rep_b, BL)
                scb = prow.tile([1, S], F16)      # score row, natural units
                scT = pwcol.tile([P, SJ], F16)    # score columns
                for st in range(ST):
                    T8 = pT.tile([P, KC, NT], FP8)
                    nc.sync.dma_start(out=T8, in_=t8_in[b * ST + st])
                    TL = pL.tile([P, KC, NT], FP8)
                    nc.sync.dma_start(out=TL, in_=tl_in[b * ST + st])

                    en = pen.tile([P, MC, NT], BF16)
                    sc = psc.tile([16, NT], F32)
                    # first-order score corrections: enc8.r_b + elo.u_b
                    for kp in range(KP):
                        nc.tensor.matmul(
                            sc,
                            lhsT=r_sb[:, 2 * kp : 2 * kp + 2, b, :],
                            rhs=T8[:, 2 * kp : 2 * kp + 2, :],
                            start=(kp == 0),
                            stop=False,
                            perf_mode=DR,
                        )
                    for kp in range(KP):
                        nc.tensor.matmul(
                            sc,
                            lhsT=u_sb[:, 2 * kp : 2 * kp + 2, b, :],
                            rhs=TL[:, 2 * kp : 2 * kp + 2, :],
                            start=False,
                            stop=False,
                            perf_mode=DR,
                        )
                    # main GEMM, fp8 DoubleRow, K=2048 PSUM-accumulated
                    for m in range(MC):
                        ps = pmm.tile([P, NT], F32)
                        for kp in range(KP):
                            nc.tensor.matmul(
                                ps,
                                lhsT=w_sb[:, 2 * kp : 2 * kp + 2, m * P : (m + 1) * P],
                                rhs=T8[:, 2 * kp : 2 * kp + 2, :],
                                start=(kp == 0),
                                stop=(kp == KP - 1),
                                perf_mode=DR,
                            )
                        nc.scalar.activation(
                            out=en[:, m, :],
                            in_=ps,
                            func=mybir.ActivationFunctionType.Tanh,
                            bias=hid_sb[:, b, m : m + 1],
                            scale=1.0 / SC_W,
                        )
                    # v-dot in bf16
                    for m in range(MC):
                        nc.tensor.matmul(
                            sc[0:1, :],
                            lhsT=v_sb[:, m : m + 1],
                            rhs=en[:, m, :],
                            start=False,
                            stop=(m == MC - 1),
                            skip_group_check=True,
                        )
                    # drain scores to fp16 row segment (natural units)
                    nc.scalar.activation(
                        out=scb[:, st * NT : (st + 1) * NT],
                        in_=sc[0:1, :],
                        func=mybir.ActivationFunctionType.Identity,
                        scale=1.0 / SC_S,
                    )
                    # fp16 transposes into score columns
                    for q in range(QT):
                        j = st * QT + q
                        pw = pwc.tile([P, 1], F16)
                        nc.tensor.transpose(pw, scb[:, j * P : (j + 1) * P], ident16)
                        nc.vector.tensor_copy(out=scT[:, j : j + 1], in_=pw)

                # softmax tail: row exp (+Z), column exp -> fp8 hi/lo weights
                exr = prow.tile([1, S], F32)
                zt = prow.tile([1, 1], F32)
                nc.scalar.activation(
                    out=exr,
                    in_=scb,
                    func=mybir.ActivationFunctionType.Exp,
                    accum_out=zt,
                )
                rz = prow.tile([1, 1], F32)
                nc.vector.reciprocal(out=rz, in_=zt)
                wrow = prow.tile([1, S], F32)
                nc.scalar.activation(
                    out=wrow,
                    in_=exr,
                    func=mybir.ActivationFunctionType.Identity,
                    scale=rz,
                )
                nc.sync.dma_start(out=attw_out[b], in_=wrow)

                wq = pwcol.tile([P, SJ], F32)
                nc.scalar.activation(
                    out=wq, in_=scT, func=mybir.ActivationFunctionType.Exp
                )
                # fp8 hi/lo exp weights, padded to 16 stationary columns
                # (dual-fp8 ldweights rejects narrower stationaries)
                w8c = pwcol.tile([P, SJ, 16], FP8)
                nc.vector.memset(w8c, 0.0)
                nc.vector.tensor_copy(out=w8c[:, :, 0], in_=wq)
                w8f = pwcol.tile([P, SJ], F32)
                nc.vector.tensor_copy(out=w8f, in_=w8c[:, :, 0])
                wlo = pwcol.tile([P, SJ, 16], FP8)
                nc.vector.memset(wlo, 0.0)
                nc.vector.tensor_sub(wlo[:, :, 0], wq, w8f)

                # context: 3-term fp8 DoubleRow over s-block pairs
                cps = [
                    pctx.tile([16, NT], F32, name=f"cps{g}_{b}_{rep}", tag=f"cps{g}")
                    for g in range(4)
                ]
                for st in range(ST):
                    n8 = pn8.tile([P, QT, E], FP8)
                    nc.sync.dma_start(out=n8, in_=n8_in[b * ST + st])
                    nl = pnl.tile([P, QT, E], FP8)
                    nc.sync.dma_start(out=nl, in_=nl_in[b * ST + st])
                    for qp in range(QT // 2):
                        jp = st * (QT // 2) + qp
                        for g in range(4):
                            for term, (lhs, rhs) in enumerate(
                                ((w8c, n8), (wlo, n8), (w8c, nl))
                            ):
                                nc.tensor.matmul(
                                    cps[g],
                                    lhsT=lhs[:, 2 * jp : 2 * jp + 2, :],
                                    rhs=rhs[
                                        :, 2 * qp : 2 * qp + 2, g * NT : (g + 1) * NT
                                    ],
                                    start=(jp == 0 and term == 0),
                                    stop=(jp == JP - 1 and term == 2),
                                    perf_mode=DR,
                                )
                ctxrow = prow.tile([1, E], F32)
                for g in range(4):
                    nc.vector.tensor_scalar_mul(
                        ctxrow[:, g * NT : (g + 1) * NT], cps[g][0:1, :], rz
                    )
                nc.sync.dma_start(out=ctx_out[b], in_=ctxrow)

    nc.compile()
    return nc


_CACHE = {}


def _get_nc():
    if "nc" not in _CACHE:
        _CACHE["nc"] = build_nc()
    return _CACHE["nc"]


def prep_in_maps(hidden, encoder_outputs, attn_w, attn_b, v_w):
    hidden = np.asarray(hidden, dtype=np.float32)
    enc = np.asarray(encoder_outputs, dtype=np.float32)
    attn_w = np.asarray(attn_w, dtype=np.float32)
    attn_b = np.asarray(attn_b, dtype=np.float32)
    v_w = np.asarray(v_w, dtype=np.float32)

    w_h = attn_w[:, :H]
    w_e = attn_w[:, H:]
    hid_proj = (hidden @ w_h.T + attn_b).astype(np.float32)   # (B, H) exact

    w8 = (w_e * SC_W).astype(FP8_NP)                           # (H, E)
    dw = w_e - w8.astype(np.float32) / SC_W                    # exact residual

    # c[b,h] = E[tanh'(hid_proj_bh + sigma_h Z)] via Gauss-Hermite
    sigma = np.linalg.norm(w_e, axis=1)
    nodes, wts = np.polynomial.hermite.hermgauss(20)
    muz = (
        hid_proj[:, :, None]
        + np.sqrt(2.0) * sigma[None, :, None] * nodes[None, None, :]
    )
    c = (wts * (1.0 - np.tanh(muz) ** 2)).sum(-1) / np.sqrt(np.pi)  # (B, H)

    u_nat = np.einsum("h,bh,he->be", v_w, c, w_e, optimize=True)
    r_nat = np.einsum("h,bh,he->be", v_w, c, dw, optimize=True)
    u8 = (u_nat * SC_S).astype(FP8_NP)                         # (B, E)
    r8 = (r_nat * SC_S).astype(FP8_NP)

    enc8 = enc.astype(FP8_NP)
    elo = (enc - enc8.astype(np.float32)).astype(FP8_NP)       # natural scale

    wT = np.ascontiguousarray(w8.T.reshape(KC, P, H).transpose(1, 0, 2))
    vcol = np.ascontiguousarray((v_w * SC_S).reshape(MC, P).T).astype(BF16_NP)

    def tlayout(x):
        # [b*ST+st, p, k, n] = x[b, st*NT+n, k*P+p]
        return np.ascontiguousarray(
            x.reshape(BL, ST, NT, KC, P).transpose(0, 1, 4, 3, 2)
        ).reshape(BL * ST, P, KC, NT)

    def nlayout(x):
        # [b*ST+st, p, q, e] = x[b, st*NT + q*P + p, e]
        return np.ascontiguousarray(
            x.reshape(BL, ST, QT, P, E).transpose(0, 1, 3, 2, 4)
        ).reshape(BL * ST, P, QT, E)

    in_maps = []
    for cix in range(NCORES):
        sl = slice(cix * BL, (cix + 1) * BL)
        hp = hid_proj[sl]
        hidc = np.ascontiguousarray(
            hp.reshape(BL, MC, P).transpose(2, 0, 1)
        ).astype(np.float32)
        rcol = np.zeros((P, KC, BL, 16), dtype=FP8_NP)
        rcol[:, :, :, 0] = r8[sl].reshape(BL, KC, P).transpose(2, 1, 0)
        ucol = np.zeros((P, KC, BL, 16), dtype=FP8_NP)
        ucol[:, :, :, 0] = u8[sl].reshape(BL, KC, P).transpose(2, 1, 0)
        in_maps.append(
            {
                "t8": tlayout(enc8[sl]),
                "tl": tlayout(elo[sl]),
                "n8": nlayout(enc8[sl]),
                "nl": nlayout(elo[sl]),
                "w8": wT,
                "v1k": vcol,
                "hidc": hidc,
                "rcol": rcol,
                "ucol": ucol,
            }
        )
    return in_maps


def kernel(hidden, encoder_outputs, attn_w, attn_b, v_w):
    in_maps = prep_in_maps(hidden, encoder_outputs, attn_w, attn_b, v_w)
    nc = _get_nc()
    res = run_bass_kernel_spmd(nc, in_maps, core_ids=list(range(NCORES)))
    ctx = np.concatenate([res.results[c]["ctx"] for c in range(NCORES)], axis=0)
    attw = np.concatenate([res.results[c]["attw"] for c in range(NCORES)], axis=0)
    return ctx.astype(np.float32), attw.astype(np.float32)


# revision 14
# speedup vs baseline: 1.1862x; 1.1157x over previous
"""Bahdanau additive-attention kernel for Trainium2, 8 NeuronCores.

Problem (B=32, S=2048, H=1024, E=2H):
    hid_proj = hidden @ w_h.T + attn_b                  # (B, H)
    enc_proj[b,s,h] = sum_e enc[b,s,e] * w_e[h,e]       # (B, S, H)   <- dominant
    energy = tanh(hid_proj[:,None,:] + enc_proj)
    scores[b,s] = sum_h energy[b,s,h] * v_w[h]
    attw = softmax(scores, axis=1)
    context[b,e] = sum_s attw[b,s] * enc[b,s,e]

Sharding: data-parallel over batch, 4 batches per core.

All heavy matmuls run in fp8e4 (TRN e4m3, max 240) with
MatmulPerfMode.DoubleRow: K=256 contracted per instruction at 0.5 PE
cycles per output row -> 2x bf16 throughput.

fp8 quantization of enc and w_e alone costs ~1.9e-2 max-rel error on the
attention weights (threshold 2e-2).  The score error is corrected to
FIRST ORDER: with pre = hid + enc@w,
  dS[b,s] ~= sum_h v_h tanh'(pre_bhs) * (elo.w_h + enc8.dw_h)
Approximating tanh'(pre_bhs) by its per-(b,h) expectation c_bh (a
Gauss-Hermite integral, since pre ~ N(hid_proj_bh, ||w_h||^2) for
unit-variance enc), the correction collapses to per-batch rank-1 GEMVs
  dS[b,s] ~= elo[b,s].u_b + enc8[b,s].r_b,  u_b = sum_h v_h c_bh w_h,
                                            r_b = sum_h v_h c_bh dw_h
whose inputs are ALL host-known -- so the whole correction row [B, S] is
precomputed exactly on the host, shipped as fp16, and added to the
drained fp16 score row with one DVE add per s-tile.  Zero device matmul
cost.  Measured (numpy sim): attw err 1.9e-2 -> 6.7e-3, ctx 2.1e-3.

The context matmul runs as a 3-term fp8 hi/lo split (exact to 2nd order
because context is linear in enc):
  ctx = (w8c + wlo).enc8 + w8c.elo,  w8c = fp8(exp(s)), wlo = fp8(exp-w8c)
with elo = fp8(enc - enc8) streamed in natural layout, giving ~0.1%
effective operand precision (better than bf16) at 3/4 the bf16 PE cost.
The v-dot stays bf16 (fp8 energy alone costs ~1e-2 attw error).

Scores are drained to fp16 rows (10-bit mantissa; bf16 would add 4e-3
error), corrected, PE-transposed as fp16 (1 cycle/row vs 2 for fp32),
and exp runs per-column on ACT for the phase-2 weights; softmax Z comes
from the row-exp's free-dim accumulator.

Scales (stored operand = natural value x scale): w8 = 256 x w_e (w values
~1e-2 would be fp8-denormal); everything else at natural scale.
Main-GEMM PSUM is 256x natural -> tanh(scale=1/256, bias=hid_proj).

All DRAM layouts are pre-transposed on the host so every DMA is a plain
contiguous per-partition copy (no xbar transpose, 8KB per partition per
descriptor).

Per-core budgets: PE 109.2 (main fp8 DR) + 27.3 (v-dot bf16) + 41
(context 3-term fp8 DR) + 3.4 (fp16 transposes) ~= 181us; DMA 3 fp8
streams (enc8 transposed, enc8 natural, elo natural) ~= 50MB ~= 140us;
ACT ~95us; DVE light.
"""

import numpy as np
import ml_dtypes

import concourse.bass as bass
import concourse.tile as tile
import concourse.mybir as mybir
from concourse import bacc
from concourse.bass_utils import run_bass_kernel_spmd

B, S, H = 32, 2048, 1024
E = 2 * H
NCORES = 8
BL = B // NCORES          # batches per core
P = 128                   # partitions
KC = E // P               # 16 contraction chunks
KP = KC // 2              # 8 DoubleRow k-pairs
MC = H // P               # 8 h chunks
NT = 512                  # moving free-dim per matmul (1 PSUM bank of fp32)
ST = S // NT              # 4 s-tiles per batch
SJ = S // P               # 16 s-128 blocks per batch
JP = SJ // 2              # 8 DoubleRow s-block pairs
QT = NT // P              # 4 s-128 blocks per s-tile

F32 = mybir.dt.float32
F16 = mybir.dt.float16
BF16 = mybir.dt.bfloat16
FP8 = mybir.dt.float8e4
BF16_NP = ml_dtypes.bfloat16
FP8_NP = ml_dtypes.float8_e4m3
DR = mybir.MatmulPerfMode.DoubleRow

SC_W = 256.0              # w8 stored scale


def build_nc(reps=1):
    nc = bacc.Bacc("TRN2", target_bir_lowering=False, debug=False)

    t8_in = nc.dram_tensor("t8", [BL * ST, P, KC, NT], FP8, kind="ExternalInput")
    n8_in = nc.dram_tensor("n8", [BL * ST, P, QT, E], FP8, kind="ExternalInput")
    nl_in = nc.dram_tensor("nl", [BL * ST, P, QT, E], FP8, kind="ExternalInput")
    w_in = nc.dram_tensor("w8", [P, KC, H], FP8, kind="ExternalInput")
    v_in = nc.dram_tensor("v1k", [P, MC], BF16, kind="ExternalInput")
    hid_in = nc.dram_tensor("hidc", [P, BL, MC], F32, kind="ExternalInput")
    corr_in = nc.dram_tensor("corr", [1, BL, S], F16, kind="ExternalInput")
    ctx_out = nc.dram_tensor("ctx", [BL, E], F32, kind="ExternalOutput")
    attw_out = nc.dram_tensor("attw", [BL, S], F32, kind="ExternalOutput")

    with tile.TileContext(nc) as tc:
        with (
            tc.tile_pool(name="singles", bufs=1) as singles,
            tc.tile_pool(name="pT", bufs=2) as pT,
            tc.tile_pool(name="pen", bufs=2) as pen,
            tc.tile_pool(name="pn8", bufs=2) as pn8,
            tc.tile_pool(name="pnl", bufs=2) as pnl,
            tc.tile_pool(name="prow", bufs=1) as prow,
            tc.tile_pool(name="pwcol", bufs=2) as pwcol,
            tc.tile_pool(name="pmm", bufs=2, space="PSUM") as pmm,
            tc.tile_pool(name="psc", bufs=1, space="PSUM") as psc,
            tc.tile_pool(name="pwc", bufs=1, space="PSUM") as pwc,
            tc.tile_pool(name="pctx", bufs=1, space="PSUM") as pctx,
        ):
            # resident small operands
            w_sb = singles.tile([P, KC, H], FP8)
            nc.sync.dma_start(out=w_sb, in_=w_in[:, :, :])
            v_sb = singles.tile([P, MC], BF16)
            nc.sync.dma_start(out=v_sb, in_=v_in[:, :])
            hid_sb = singles.tile([P, BL, MC], F32)
            nc.sync.dma_start(out=hid_sb, in_=hid_in[:, :, :])
            corr_sb = singles.tile([1, BL, S], F16)
            nc.sync.dma_start(out=corr_sb, in_=corr_in[:, :, :])
            ident16 = singles.tile([1, 1], F16)
            nc.vector.memset(ident16, 1.0)

            for rep_b in range(reps * BL):
                rep, b = divmod(rep_b, BL)
                scb = prow.tile([1, S], F16)      # corrected score row
                scT = pwcol.tile([P, SJ], F16)    # score columns
                for st in range(ST):
                    T8 = pT.tile([P, KC, NT], FP8)
                    nc.sync.dma_start(out=T8, in_=t8_in[b * ST + st])

                    en = pen.tile([P, MC, NT], BF16)
                    sc = psc.tile([1, NT], F32)
                    # main GEMM, fp8 DoubleRow, K=2048 PSUM-accumulated
                    for m in range(MC):
                        ps = pmm.tile([P, NT], F32)
                        for kp in range(KP):
                            nc.tensor.matmul(
                                ps,
                                lhsT=w_sb[:, 2 * kp : 2 * kp + 2, m * P : (m + 1) * P],
                                rhs=T8[:, 2 * kp : 2 * kp + 2, :],
                                start=(kp == 0),
                                stop=(kp == KP - 1),
                                perf_mode=DR,
                            )
                        nc.scalar.activation(
                            out=en[:, m, :],
                            in_=ps,
                            func=mybir.ActivationFunctionType.Tanh,
                            bias=hid_sb[:, b, m : m + 1],
                            scale=1.0 / SC_W,
                        )
                    # v-dot in bf16
                    for m in range(MC):
                        nc.tensor.matmul(
                            sc,
                            lhsT=v_sb[:, m : m + 1],
                            rhs=en[:, m, :],
                            start=(m == 0),
                            stop=(m == MC - 1),
                        )
                    # drain + add the host-precomputed first-order correction
                    scr = prow.tile([1, NT], F16, name=f"scr{rep_b}_{st}", tag="scr")
                    nc.scalar.activation(
                        out=scr,
                        in_=sc,
                        func=mybir.ActivationFunctionType.Identity,
                    )
                    nc.vector.tensor_add(
                        out=scb[:, st * NT : (st + 1) * NT],
                        in0=scr,
                        in1=corr_sb[:, b, st * NT : (st + 1) * NT],
                    )
                    # fp16 transposes into score columns
                    for q in range(QT):
                        j = st * QT + q
                        pw = pwc.tile([P, 1], F16)
                        nc.tensor.transpose(pw, scb[:, j * P : (j + 1) * P], ident16)
                        nc.vector.tensor_copy(out=scT[:, j : j + 1], in_=pw)

                # softmax tail: row exp (+Z), column exp -> fp8 hi/lo weights
                exr = prow.tile([1, S], F32)
                zt = prow.tile([1, 1], F32)
                nc.scalar.activation(
                    out=exr,
                    in_=scb,
                    func=mybir.ActivationFunctionType.Exp,
                    accum_out=zt,
                )
                rz = prow.tile([1, 1], F32)
                nc.vector.reciprocal(out=rz, in_=zt)
                wrow = prow.tile([1, S], F32)
                nc.scalar.activation(
                    out=wrow,
                    in_=exr,
                    func=mybir.ActivationFunctionType.Identity,
                    scale=rz,
                )
                nc.sync.dma_start(out=attw_out[b], in_=wrow)

                wq = pwcol.tile([P, SJ], F32)
                nc.scalar.activation(
                    out=wq, in_=scT, func=mybir.ActivationFunctionType.Exp
                )
                # fp8 hi/lo exp weights, padded to 16 stationary columns
                # (dual-fp8 ldweights rejects narrower stationaries)
                w8c = pwcol.tile([P, SJ, 16], FP8)
                nc.vector.memset(w8c, 0.0)
                nc.vector.tensor_copy(out=w8c[:, :, 0], in_=wq)
                w8f = pwcol.tile([P, SJ], F32)
                nc.vector.tensor_copy(out=w8f, in_=w8c[:, :, 0])
                wlo = pwcol.tile([P, SJ, 16], FP8)
                nc.vector.memset(wlo, 0.0)
                nc.vector.tensor_sub(wlo[:, :, 0], wq, w8f)

                # context: 3-term fp8 DoubleRow over s-block pairs
                cps = [
                    pctx.tile([16, NT], F32, name=f"cps{g}_{b}_{rep}", tag=f"cps{g}")
                    for g in range(4)
                ]
                for st in range(ST):
                    n8 = pn8.tile([P, QT, E], FP8)
                    nc.sync.dma_start(out=n8, in_=n8_in[b * ST + st])
                    nl = pnl.tile([P, QT, E], FP8)
                    nc.sync.dma_start(out=nl, in_=nl_in[b * ST + st])
                    for qp in range(QT // 2):
                        jp = st * (QT // 2) + qp
                        for g in range(4):
                            for term, (lhs, rhs) in enumerate(
                                ((w8c, n8), (wlo, n8), (w8c, nl))
                            ):
                                nc.tensor.matmul(
                                    cps[g],
                                    lhsT=lhs[:, 2 * jp : 2 * jp + 2, :],
                                    rhs=rhs[
                                        :, 2 * qp : 2 * qp + 2, g * NT : (g + 1) * NT
                                    ],
                                    start=(jp == 0 and term == 0),
                                    stop=(jp == JP - 1 and term == 2),
                                    perf_mode=DR,
                                )
                ctxrow = prow.tile([1, E], F32)
                for g in range(4):
                    nc.vector.tensor_scalar_mul(
                        ctxrow[:, g * NT : (g + 1) * NT], cps[g][0:1, :], rz
                    )
                nc.sync.dma_start(out=ctx_out[b], in_=ctxrow)

    nc.compile()
    return nc


_CACHE = {}


def _get_nc():
    if "nc" not in _CACHE:
        _CACHE["nc"] = build_nc()
    return _CACHE["nc"]


def prep_in_maps(hidden, encoder_outputs, attn_w, attn_b, v_w):
    hidden = np.asarray(hidden, dtype=np.float32)
    enc = np.asarray(encoder_outputs, dtype=np.float32)
    attn_w = np.asarray(attn_w, dtype=np.float32)
    attn_b = np.asarray(attn_b, dtype=np.float32)
    v_w = np.asarray(v_w, dtype=np.float32)

    w_h = attn_w[:, :H]
    w_e = attn_w[:, H:]
    hid_proj = (hidden @ w_h.T + attn_b).astype(np.float32)   # (B, H) exact

    w8 = (w_e * SC_W).astype(FP8_NP)                           # (H, E)
    dw = w_e - w8.astype(np.float32) / SC_W                    # exact residual

    # c[b,h] = E[tanh'(hid_proj_bh + sigma_h Z)] via Gauss-Hermite
    sigma = np.linalg.norm(w_e, axis=1)
    nodes, wts = np.polynomial.hermite.hermgauss(20)
    muz = (
        hid_proj[:, :, None]
        + np.sqrt(2.0) * sigma[None, :, None] * nodes[None, None, :]
    )
    c = (wts * (1.0 - np.tanh(muz) ** 2)).sum(-1) / np.sqrt(np.pi)  # (B, H)

    u_nat = np.einsum("h,bh,he->be", v_w, c, w_e, optimize=True)
    r_nat = np.einsum("h,bh,he->be", v_w, c, dw, optimize=True)

    enc8 = enc.astype(FP8_NP)
    enc8f = enc8.astype(np.float32)
    elo = (enc - enc8f).astype(FP8_NP)                         # natural scale

    # full first-order score correction, exact on the host:
    # corr[b,s] = (enc - enc8)[b,s] . u_b  +  enc8[b,s] . r_b
    corr = (
        np.einsum("bse,be->bs", enc - enc8f, u_nat, optimize=True)
        + np.einsum("bse,be->bs", enc8f, r_nat, optimize=True)
    ).astype(np.float16)                                       # (B, S)

    wT = np.ascontiguousarray(w8.T.reshape(KC, P, H).transpose(1, 0, 2))
    vcol = np.ascontiguousarray(v_w.reshape(MC, P).T).astype(BF16_NP)

    def tlayout(x):
        # [b*ST+st, p, k, n] = x[b, st*NT+n, k*P+p]
        return np.ascontiguousarray(
            x.reshape(BL, ST, NT, KC, P).transpose(0, 1, 4, 3, 2)
        ).reshape(BL * ST, P, KC, NT)

    def nlayout(x):
        # [b*ST+st, p, q, e] = x[b, st*NT + q*P + p, e]
        return np.ascontiguousarray(
            x.reshape(BL, ST, QT, P, E).transpose(0, 1, 3, 2, 4)
        ).reshape(BL * ST, P, QT, E)

    in_maps = []
    for cix in range(NCORES):
        sl = slice(cix * BL, (cix + 1) * BL)
        hp = hid_proj[sl]
        hidc = np.ascontiguousarray(
            hp.reshape(BL, MC, P).transpose(2, 0, 1)
        ).astype(np.float32)
        in_maps.append(
            {
                "t8": tlayout(enc8[sl]),
                "n8": nlayout(enc8[sl]),
                "nl": nlayout(elo[sl]),
                "w8": wT,
                "v1k": vcol,
                "hidc": hidc,
                "corr": corr[sl][None, :, :],
            }
        )
    return in_maps


def kernel(hidden, encoder_outputs, attn_w, attn_b, v_w):
    in_maps = prep_in_maps(hidden, encoder_outputs, attn_w, attn_b, v_w)
    nc = _get_nc()
    res = run_bass_kernel_spmd(nc, in_maps, core_ids=list(range(NCORES)))
    ctx = np.concatenate([res.results[c]["ctx"] for c in range(NCORES)], axis=0)
    attw = np.concatenate([res.results[c]["attw"] for c in range(NCORES)], axis=0)
    return ctx.astype(np.float32), attw.astype(np.float32)


# revision 15
# speedup vs baseline: 1.2217x; 1.0299x over previous
"""Bahdanau additive-attention kernel for Trainium2, 8 NeuronCores.

Problem (B=32, S=2048, H=1024, E=2H):
    hid_proj = hidden @ w_h.T + attn_b                  # (B, H)
    enc_proj[b,s,h] = sum_e enc[b,s,e] * w_e[h,e]       # (B, S, H)   <- dominant
    energy = tanh(hid_proj[:,None,:] + enc_proj)
    scores[b,s] = sum_h energy[b,s,h] * v_w[h]
    attw = softmax(scores, axis=1)
    context[b,e] = sum_s attw[b,s] * enc[b,s,e]

Sharding: data-parallel over batch, 4 batches per core.

All heavy matmuls run in fp8e4 (TRN e4m3, max 240) with
MatmulPerfMode.DoubleRow: K=256 contracted per instruction at 0.5 PE
cycles per output row -> 2x bf16 throughput.

fp8 quantization of enc and w_e alone costs ~1.9e-2 max-rel error on the
attention weights (threshold 2e-2).  The score error is corrected to
FIRST ORDER: with pre = hid + enc@w,
  dS[b,s] ~= sum_h v_h tanh'(pre_bhs) * (elo.w_h + enc8.dw_h)
Approximating tanh'(pre_bhs) by its per-(b,h) expectation c_bh (a
Gauss-Hermite integral, since pre ~ N(hid_proj_bh, ||w_h||^2) for
unit-variance enc), the correction collapses to per-batch rank-1 GEMVs
  dS[b,s] ~= elo[b,s].u_b + enc8[b,s].r_b,  u_b = sum_h v_h c_bh w_h,
                                            r_b = sum_h v_h c_bh dw_h
whose inputs are ALL host-known -- so the whole correction row [B, S] is
precomputed exactly on the host, shipped as fp16, and added to the
drained fp16 score row with one DVE add per s-tile.  Zero device matmul
cost.  Measured (numpy sim): attw err 1.9e-2 -> 6.7e-3, ctx 2.1e-3.

The context matmul runs as a 3-term fp8 hi/lo split (exact to 2nd order
because context is linear in enc):
  ctx = (w8c + wlo).enc8 + w8c.elo,  w8c = fp8(exp(s)), wlo = fp8(exp-w8c)
with elo = fp8(enc - enc8) streamed in natural layout, giving ~0.1%
effective operand precision (better than bf16) at 3/4 the bf16 PE cost.
The v-dot stays bf16 (fp8 energy alone costs ~1e-2 attw error).

Scores are drained to fp16 rows (10-bit mantissa; bf16 would add 4e-3
error), corrected, PE-transposed as fp16 (1 cycle/row vs 2 for fp32),
and exp runs per-column on ACT for the phase-2 weights; softmax Z comes
from the row-exp's free-dim accumulator.

Scales (stored operand = natural value x scale): w8 = 256 x w_e (w values
~1e-2 would be fp8-denormal); everything else at natural scale.
Main-GEMM PSUM is 256x natural -> tanh(scale=1/256, bias=hid_proj).

All DRAM layouts are pre-transposed on the host so every DMA is a plain
contiguous per-partition copy (no xbar transpose, 8KB per partition per
descriptor).

Per-core budgets: PE 109.2 (main fp8 DR) + 27.3 (v-dot bf16) + 41
(context 3-term fp8 DR) + 3.4 (fp16 transposes) ~= 181us; DMA 3 fp8
streams (enc8 transposed, enc8 natural, elo natural) ~= 50MB ~= 140us;
ACT ~95us; DVE light.
"""

import numpy as np
import ml_dtypes

import concourse.bass as bass
import concourse.tile as tile
import concourse.mybir as mybir
from concourse import bacc
from concourse.bass_utils import run_bass_kernel_spmd

B, S, H = 32, 2048, 1024
E = 2 * H
NCORES = 8
BL = B // NCORES          # batches per core
P = 128                   # partitions
KC = E // P               # 16 contraction chunks
KP = KC // 2              # 8 DoubleRow k-pairs
MC = H // P               # 8 h chunks
NT = 512                  # moving free-dim per matmul (1 PSUM bank of fp32)
ST = S // NT              # 4 s-tiles per batch
SJ = S // P               # 16 s-128 blocks per batch
JP = SJ // 2              # 8 DoubleRow s-block pairs
QT = NT // P              # 4 s-128 blocks per s-tile

F32 = mybir.dt.float32
F16 = mybir.dt.float16
BF16 = mybir.dt.bfloat16
FP8 = mybir.dt.float8e4
BF16_NP = ml_dtypes.bfloat16
FP8_NP = ml_dtypes.float8_e4m3
DR = mybir.MatmulPerfMode.DoubleRow

SC_W = 256.0              # w8 stored scale


def build_nc(reps=1):
    nc = bacc.Bacc("TRN2", target_bir_lowering=False, debug=False)

    t8_in = nc.dram_tensor("t8", [BL * ST, P, KC, NT], FP8, kind="ExternalInput")
    n8_in = nc.dram_tensor("n8", [BL * ST, P, QT, E], FP8, kind="ExternalInput")
    nl_in = nc.dram_tensor("nl", [BL * ST, P, QT, E], FP8, kind="ExternalInput")
    w_in = nc.dram_tensor("w8", [P, KC, H], FP8, kind="ExternalInput")
    v_in = nc.dram_tensor("v1k", [P, MC], BF16, kind="ExternalInput")
    hid_in = nc.dram_tensor("hidc", [P, BL, MC], F32, kind="ExternalInput")
    corr_in = nc.dram_tensor("corr", [1, BL, S], F16, kind="ExternalInput")
    ctx_out = nc.dram_tensor("ctx", [BL, E], F32, kind="ExternalOutput")
    attw_out = nc.dram_tensor("attw", [BL, S], F32, kind="ExternalOutput")

    with tile.TileContext(nc) as tc:
        with (
            tc.tile_pool(name="singles", bufs=1) as singles,
            tc.tile_pool(name="pT", bufs=2) as pT,
            tc.tile_pool(name="pen", bufs=2) as pen,
            tc.tile_pool(name="pn8", bufs=2) as pn8,
            tc.tile_pool(name="pnl", bufs=2) as pnl,
            tc.tile_pool(name="prow", bufs=1) as prow,
            tc.tile_pool(name="pwcol", bufs=2) as pwcol,
            tc.tile_pool(name="pmm", bufs=2, space="PSUM") as pmm,
            tc.tile_pool(name="psc", bufs=1, space="PSUM") as psc,
            tc.tile_pool(name="pwc", bufs=1, space="PSUM") as pwc,
            tc.tile_pool(name="pctx", bufs=1, space="PSUM") as pctx,
        ):
            # resident small operands
            w_sb = singles.tile([P, KC, H], FP8)
            nc.sync.dma_start(out=w_sb, in_=w_in[:, :, :])
            v_sb = singles.tile([P, MC], BF16)
            nc.sync.dma_start(out=v_sb, in_=v_in[:, :])
            hid_sb = singles.tile([P, BL, MC], F32)
            nc.sync.dma_start(out=hid_sb, in_=hid_in[:, :, :])
            corr_sb = singles.tile([1, BL, S], F16)
            nc.sync.dma_start(out=corr_sb, in_=corr_in[:, :, :])
            ident16 = singles.tile([1, 1], F16)
            nc.vector.memset(ident16, 1.0)

            for rep_b in range(reps * BL):
                rep, b = divmod(rep_b, BL)
                scb = prow.tile([1, S], F16)      # corrected score row
                scT = pwcol.tile([P, SJ], F16)    # score columns
                # fp8 hi/lo exp weights, padded to 16 stationary columns
                # (dual-fp8 ldweights rejects narrower stationaries);
                # memsets run at batch start, segments fill per s-tile
                wq = pwcol.tile([P, SJ], F32)
                w8f = pwcol.tile([P, SJ], F32)
                w8c = pwcol.tile([P, SJ, 16], FP8)
                nc.vector.memset(w8c, 0.0)
                wlo = pwcol.tile([P, SJ, 16], FP8)
                nc.vector.memset(wlo, 0.0)
                for st in range(ST):
                    T8 = pT.tile([P, KC, NT], FP8)
                    nc.sync.dma_start(out=T8, in_=t8_in[b * ST + st])

                    en = pen.tile([P, MC, NT], BF16)
                    sc = psc.tile([1, NT], F32)
                    # main GEMM, fp8 DoubleRow, K=2048 PSUM-accumulated
                    for m in range(MC):
                        ps = pmm.tile([P, NT], F32)
                        for kp in range(KP):
                            nc.tensor.matmul(
                                ps,
                                lhsT=w_sb[:, 2 * kp : 2 * kp + 2, m * P : (m + 1) * P],
                                rhs=T8[:, 2 * kp : 2 * kp + 2, :],
                                start=(kp == 0),
                                stop=(kp == KP - 1),
                                perf_mode=DR,
                            )
                        nc.scalar.activation(
                            out=en[:, m, :],
                            in_=ps,
                            func=mybir.ActivationFunctionType.Tanh,
                            bias=hid_sb[:, b, m : m + 1],
                            scale=1.0 / SC_W,
                        )
                    # v-dot in bf16
                    for m in range(MC):
                        nc.tensor.matmul(
                            sc,
                            lhsT=v_sb[:, m : m + 1],
                            rhs=en[:, m, :],
                            start=(m == 0),
                            stop=(m == MC - 1),
                        )
                    # drain + add the host-precomputed first-order correction
                    scr = prow.tile([1, NT], F16, name=f"scr{rep_b}_{st}", tag="scr")
                    nc.scalar.activation(
                        out=scr,
                        in_=sc,
                        func=mybir.ActivationFunctionType.Identity,
                    )
                    nc.vector.tensor_add(
                        out=scb[:, st * NT : (st + 1) * NT],
                        in0=scr,
                        in1=corr_sb[:, b, st * NT : (st + 1) * NT],
                    )
                    # fp16 transposes into score columns
                    for q in range(QT):
                        j = st * QT + q
                        pw = pwc.tile([P, 1], F16)
                        nc.tensor.transpose(pw, scb[:, j * P : (j + 1) * P], ident16)
                        nc.vector.tensor_copy(out=scT[:, j : j + 1], in_=pw)
                    sq = slice(st * QT, (st + 1) * QT)
                    nc.scalar.activation(
                        out=wq[:, sq],
                        in_=scT[:, sq],
                        func=mybir.ActivationFunctionType.Exp,
                    )
                    nc.vector.tensor_copy(out=w8c[:, sq, 0], in_=wq[:, sq])
                    nc.vector.tensor_copy(out=w8f[:, sq], in_=w8c[:, sq, 0])
                    nc.vector.tensor_sub(wlo[:, sq, 0], wq[:, sq], w8f[:, sq])

                # softmax tail: row exp (+Z), column exp -> fp8 hi/lo weights
                exr = prow.tile([1, S], F32)
                zt = prow.tile([1, 1], F32)
                nc.scalar.activation(
                    out=exr,
                    in_=scb,
                    func=mybir.ActivationFunctionType.Exp,
                    accum_out=zt,
                )
                rz = prow.tile([1, 1], F32)
                nc.vector.reciprocal(out=rz, in_=zt)
                wrow = prow.tile([1, S], F32)
                nc.scalar.activation(
                    out=wrow,
                    in_=exr,
                    func=mybir.ActivationFunctionType.Identity,
                    scale=rz,
                )
                nc.sync.dma_start(out=attw_out[b], in_=wrow)

                # context: 3-term fp8 DoubleRow over s-block pairs
                cps = [
                    pctx.tile([16, NT], F32, name=f"cps{g}_{b}_{rep}", tag=f"cps{g}")
                    for g in range(4)
                ]
                for st in range(ST):
                    n8 = pn8.tile([P, QT, E], FP8)
                    nc.sync.dma_start(out=n8, in_=n8_in[b * ST + st])
                    nl = pnl.tile([P, QT, E], FP8)
                    nc.sync.dma_start(out=nl, in_=nl_in[b * ST + st])
                    for qp in range(QT // 2):
                        jp = st * (QT // 2) + qp
                        for g in range(4):
                            for term, (lhs, rhs) in enumerate(
                                ((w8c, n8), (wlo, n8), (w8c, nl))
                            ):
                                nc.tensor.matmul(
                                    cps[g],
                                    lhsT=lhs[:, 2 * jp : 2 * jp + 2, :],
                                    rhs=rhs[
                                        :, 2 * qp : 2 * qp + 2, g * NT : (g + 1) * NT
                                    ],
                                    start=(jp == 0 and term == 0),
                                    stop=(jp == JP - 1 and term == 2),
                                    perf_mode=DR,
                                )
                ctxrow = prow.tile([1, E], F32)
                for g in range(4):
                    nc.vector.tensor_scalar_mul(
                        ctxrow[:, g * NT : (g + 1) * NT], cps[g][0:1, :], rz
                    )
                nc.sync.dma_start(out=ctx_out[b], in_=ctxrow)

    nc.compile()
    return nc


_CACHE = {}


def _get_nc():
    if "nc" not in _CACHE:
        _CACHE["nc"] = build_nc()
    return _CACHE["nc"]


def prep_in_maps(hidden, encoder_outputs, attn_w, attn_b, v_w):
    hidden = np.asarray(hidden, dtype=np.float32)
    enc = np.asarray(encoder_outputs, dtype=np.float32)
    attn_w = np.asarray(attn_w, dtype=np.float32)
    attn_b = np.asarray(attn_b, dtype=np.float32)
    v_w = np.asarray(v_w, dtype=np.float32)

    w_h = attn_w[:, :H]
    w_e = attn_w[:, H:]
    hid_proj = (hidden @ w_h.T + attn_b).astype(np.float32)   # (B, H) exact

    w8 = (w_e * SC_W).astype(FP8_NP)                           # (H, E)
    dw = w_e - w8.astype(np.float32) / SC_W                    # exact residual

    # c[b,h] = E[tanh'(hid_proj_bh + sigma_h Z)] via Gauss-Hermite
    sigma = np.linalg.norm(w_e, axis=1)
    nodes, wts = np.polynomial.hermite.hermgauss(20)
    muz = (
        hid_proj[:, :, None]
        + np.sqrt(2.0) * sigma[None, :, None] * nodes[None, None, :]
    )
    c = (wts * (1.0 - np.tanh(muz) ** 2)).sum(-1) / np.sqrt(np.pi)  # (B, H)

    u_nat = np.einsum("h,bh,he->be", v_w, c, w_e, optimize=True)
    r_nat = np.einsum("h,bh,he->be", v_w, c, dw, optimize=True)

    enc8 = enc.astype(FP8_NP)
    enc8f = enc8.astype(np.float32)
    elo = (enc - enc8f).astype(FP8_NP)                         # natural scale

    # full first-order score correction, exact on the host:
    # corr[b,s] = (enc - enc8)[b,s] . u_b  +  enc8[b,s] . r_b
    corr = (
        np.einsum("bse,be->bs", enc - enc8f, u_nat, optimize=True)
        + np.einsum("bse,be->bs", enc8f, r_nat, optimize=True)
    ).astype(np.float16)                                       # (B, S)

    wT = np.ascontiguousarray(w8.T.reshape(KC, P, H).transpose(1, 0, 2))
    vcol = np.ascontiguousarray(v_w.reshape(MC, P).T).astype(BF16_NP)

    def tlayout(x):
        # [b*ST+st, p, k, n] = x[b, st*NT+n, k*P+p]
        return np.ascontiguousarray(
            x.reshape(BL, ST, NT, KC, P).transpose(0, 1, 4, 3, 2)
        ).reshape(BL * ST, P, KC, NT)

    def nlayout(x):
        # [b*ST+st, p, q, e] = x[b, st*NT + q*P + p, e]
        return np.ascontiguousarray(
            x.reshape(BL, ST, QT, P, E).transpose(0, 1, 3, 2, 4)
        ).reshape(BL * ST, P, QT, E)

    in_maps = []
    for cix in range(NCORES):
        sl = slice(cix * BL, (cix + 1) * BL)
        hp = hid_proj[sl]
        hidc = np.ascontiguousarray(
            hp.reshape(BL, MC, P).transpose(2, 0, 1)
        ).astype(np.float32)
        in_maps.append(
            {
                "t8": tlayout(enc8[sl]),
                "n8": nlayout(enc8[sl]),
                "nl": nlayout(elo[sl]),
                "w8": wT,
                "v1k": vcol,
                "hidc": hidc,
                "corr": corr[sl][None, :, :],
            }
        )
    return in_maps


def kernel(hidden, encoder_outputs, attn_w, attn_b, v_w):
    in_maps = prep_in_maps(hidden, encoder_outputs, attn_w, attn_b, v_w)
    nc = _get_nc()
    res = run_bass_kernel_spmd(nc, in_maps, core_ids=list(range(NCORES)))
    ctx = np.concatenate([res.results[c]["ctx"] for c in range(NCORES)], axis=0)
    attw = np.concatenate([res.results[c]["attw"] for c in range(NCORES)], axis=0)
    return ctx.astype(np.float32), attw.astype(np.float32)
